# revision 5
# baseline (speedup 1.0000x reference)
"""MoE routing kernel for Trainium2 (8 NeuronCores, data-parallel over batch).

Stage-1 matmul in fp16 (4x PE rate vs fp32, half the HBM bytes
for the dominant x^T stream, pre-converted host-side); gather stream
(pN) and top-8 mixing weights also fp16. Softmax/top-8 selection stays
f32. 8-chunk DMA groups give 2-4KB contiguous runs per partition.
Tile mix (128,256,256,256,128) spreads the W1 load crunch and the
per-tile finish chains evenly. Each tile's stage-2/3 + routing is
deferred until the next tile's first stage-1 groups are enqueued, so
the in-order PE queue never stalls on the scalar relu at tile
boundaries.

Gather: one SWDGE dma_gather per 128-row subtile fetches all 8
selected expert rows for all 128 rows (1024 descriptors) in a single
instruction — SWDGE cost is ~994ns fixed + 0.34ns/desc, so this is
~8x cheaper than 8 per-slot indirect DMAs (which serialized at
~1.4us each on the gpsimd queue and dominated the kernel tail).
dma_gather wants int16 row indices wrapped 16-partition-style
(idx i at [i%16, i//16]); indices are subtile-local (p*64+e < 8192,
int16-safe, gather base = pN + bg*8192 rows) and the wrap layout is
produced with a 2KB DRAM round-trip (SBUF [128,8] -> DRAM flat
j*128+p order -> SBUF [16,64]), both legs on the DVE HWDGE queue so
the sync queue (x-stream) and gpsimd queue (gathers) never stall.

Pipeline per core (batch shard of 1024 rows):
  h1^T = relu(W1^T @ flat^T + b1)   # contraction D=16384, fp16 on PE
  h2^T = relu(W2^T @ h1^T + b2)
  logits = h2^T.T @ W3 + b3         # [128b, 64e] tiles
  s = softmax(logits) in f32; top-8 via DVE max/max_index;
  weights = top_vals / sum(top_vals)
  dma_gather selected expert rows; weighted sum; store.

Host-side layout: pTr[p, (t,g,cc,b)] so each (tile, k-group) DMA is a
single [128 x kper*nt] straight copy with kper*nt*2B contiguous runs
per partition (4 KB for the 256-wide tiles).
"""

import numpy as np

B, E, C, TOPK = 8192, 64, 256, 8
D, H1, H2 = 16384, 256, 128
NCORES = 8
BS = B // NCORES  # batch rows per core
P = 128
KPER = 8
TILES = (128, 256, 256, 256, 128)

_BUILD_CACHE = {}


def build_moe_nc(bs=BS, d=D, e=E, c=C, h1=H1, h2=H2, tiles=TILES, kper=KPER,
                 xbufs=12, hbufs=3, use_dma_gather=True, debug_taps=False):
    import concourse.bacc as bacc
    import concourse.bass as bass
    import concourse.mybir as mybir
    from concourse import tile

    f32 = mybir.dt.float32
    f16 = mybir.dt.float16
    u32 = mybir.dt.uint32
    i16 = mybir.dt.int16
    KC = d // P            # 128-row K-chunks in main contraction
    KG = KC // kper        # DMA groups of kper chunks
    MC = h1 // P           # output row chunks of h1^T
    assert sum(tiles) == bs
    KC2 = h1 // P          # K-chunks for stage 2
    NBT = bs // P          # total 128-row batch subtiles

    nc = bacc.Bacc("TRN2", target_bir_lowering=False, debug=False,
                   num_devices=NCORES)

    pTr = nc.dram_tensor("pTr", [P, KC * bs], f16, kind="ExternalInput").ap()
    pN = nc.dram_tensor("pN", [bs * e, c], f16, kind="ExternalInput").ap()
    w1r = nc.dram_tensor("w1r", [P, KC * h1], f16, kind="ExternalInput").ap()
    w2r = nc.dram_tensor("w2r", [P, KC2 * h2], f32, kind="ExternalInput").ap()
    w3 = nc.dram_tensor("w3", [h2, e], f32, kind="ExternalInput").ap()
    b1r = nc.dram_tensor("b1r", [P, MC], f32, kind="ExternalInput").ap()
    b2r = nc.dram_tensor("b2r", [P, 1], f32, kind="ExternalInput").ap()
    b3r = nc.dram_tensor("b3r", [P, e], f32, kind="ExternalInput").ap()
    out = nc.dram_tensor("out", [bs, c], f16, kind="ExternalOutput").ap()
    if debug_taps:
        dbg_lg = nc.dram_tensor("dbg_lg", [bs, e], f32, kind="ExternalOutput").ap()
        dbg_ti = nc.dram_tensor("dbg_ti", [bs, 8], u32, kind="ExternalOutput").ap()
        dbg_w = nc.dram_tensor("dbg_w", [bs, 8], f32, kind="ExternalOutput").ap()

    AF = mybir.ActivationFunctionType
    OP = mybir.AluOpType

    with tile.TileContext(nc) as tc:
        with (
            tc.tile_pool(name="wconst", bufs=1) as wconst,
            tc.tile_pool(name="w1pool", bufs=1) as w1pool,
            tc.tile_pool(name="xpool", bufs=xbufs) as xpool,
            tc.tile_pool(name="hpool", bufs=hbufs) as hpool,
            tc.tile_pool(name="spool", bufs=6) as spool,
            tc.tile_pool(name="selpool", bufs=4) as selpool,
            tc.tile_pool(name="ipool", bufs=2) as ipool,
            tc.tile_pool(name="dpool", bufs=2, space="DRAM") as dpool,
            tc.tile_pool(name="opool", bufs=1) as opool,
            tc.tile_pool(name="psh1", bufs=2, space="PSUM") as psh1,
            tc.tile_pool(name="psh2", bufs=2, space="PSUM") as psh2,
            tc.tile_pool(name="pslg", bufs=2, space="PSUM") as pslg,
        ):
            # --- constants (small); emitted AFTER the first k-group DMAs so
            # they don't delay the PE-critical xt/W1 stream at startup
            cst = {}

            def emit_consts():
                cst["w2"] = wconst.tile([P, KC2 * h2], f32, name="w2_sb")
                nc.scalar.dma_start(out=cst["w2"], in_=w2r)
                cst["w3"] = wconst.tile([P, e], f32, name="w3_sb")
                nc.scalar.dma_start(out=cst["w3"][:h2, :], in_=w3)
                cst["b1"] = wconst.tile([P, MC], f32, name="b1_sb")
                nc.scalar.dma_start(out=cst["b1"], in_=b1r)
                cst["b2"] = wconst.tile([P, 1], f32, name="b2_sb")
                nc.scalar.dma_start(out=cst["b2"], in_=b2r)
                cst["b3"] = wconst.tile([P, e], f32, name="b3_sb")
                nc.scalar.dma_start(out=cst["b3"], in_=b3r)
                if use_dma_gather:
                    # rb64[p] = p*64: subtile-local DRAM row base per partition
                    rb = wconst.tile([P, 1], u32, name="rb64")
                    nc.gpsimd.iota(rb, pattern=[[0, 1]], base=0,
                                   channel_multiplier=e)
                    cst["rb64"] = rb
                else:
                    for bg in range(NBT):
                        rb = wconst.tile([P, 1], u32, tag=f"rb_{bg}",
                                         name=f"rb_{bg}")
                        nc.gpsimd.iota(rb, pattern=[[0, 1]], base=bg * P * e,
                                       channel_multiplier=e)
                        cst[f"rb_{bg}"] = rb

            # --- W1 group tiles: persistent, loaded just-in-time in n=0 loop
            w1_tiles = [None] * KG
            acc_tiles = []

            def finish_tile(nt, col0, ps_h1):
                # relu(h1^T + b1) -> SBUF (f32: stage 2/3 stay full precision)
                h1r = []
                for m in range(MC):
                    hr = hpool.tile([P, nt], f32, tag=f"h1r_{m}", name=f"h1r_{m}")
                    nc.scalar.activation(hr, ps_h1[m], AF.Relu,
                                         bias=cst["b1"][:, m:m + 1])
                    h1r.append(hr)

                # h2^T
                ps_h2 = psh2.tile([P, nt], f32, tag="h2", name="ps_h2")
                for k2 in range(KC2):
                    nc.tensor.matmul(out=ps_h2[:h2, :],
                                     lhsT=cst["w2"][:, k2 * h2:(k2 + 1) * h2],
                                     rhs=h1r[k2], start=(k2 == 0),
                                     stop=(k2 == KC2 - 1))
                h2r = hpool.tile([P, nt], f32, tag="h2r", name="h2r")
                nc.scalar.activation(h2r[:h2, :], ps_h2[:h2, :], AF.Relu,
                                     bias=cst["b2"][:h2, :])

                for bt in range(nt // P):
                    bg = col0 // P + bt  # global 128-row batch subtile index
                    ps_lg = pslg.tile([P, e], f32, tag="lg", name="ps_lg")
                    nc.tensor.matmul(out=ps_lg, lhsT=h2r[:h2, bt * P:(bt + 1) * P],
                                     rhs=cst["w3"][:h2, :], start=True, stop=True)
                    lg = spool.tile([P, e], f32, tag="lg_sb", name="lg_sb")
                    nc.vector.tensor_tensor(out=lg, in0=ps_lg, in1=cst["b3"], op=OP.add)

                    # f32 softmax, replicating the reference's quantization
                    nm = spool.tile([P, 1], f32, tag="nm", name="nm")
                    nc.vector.reduce_max(out=nm, in_=lg, axis=mybir.AxisListType.X,
                                         negate=True)
                    ef = spool.tile([P, e], f32, tag="ef", name="ef")
                    nc.scalar.activation(ef, lg, AF.Exp, bias=nm)
                    # top-8 straight on the unnormalized exps: selection
                    # order is scale-invariant and tv/sum(tv) below cancels
                    # the softmax normalization
                    tv = spool.tile([P, 8], f32, tag="tv", name="tv")
                    nc.vector.max(out=tv, in_=ef)
                    ti = spool.tile([P, 8], u32, tag="ti", name="ti")
                    nc.vector.max_index(out=ti, in_max=tv, in_values=ef)

                    s8 = spool.tile([P, 1], f32, tag="s8", name="s8")
                    nc.vector.reduce_sum(out=s8, in_=tv, axis=mybir.AxisListType.X)
                    r8 = spool.tile([P, 1], f32, tag="r8", name="r8")
                    nc.vector.reciprocal(r8, s8)
                    wgt = spool.tile([P, 8], f16, tag="wgt", name="wgt")
                    nc.scalar.activation(wgt, tv, AF.Copy, scale=r8)

                    sel = selpool.tile([P, TOPK, c], f16, tag="sel", name="sel")
                    mt = selpool.tile([P, TOPK * c], f16, tag="mt", name="mt")
                    mt3 = mt.rearrange("p (k c) -> p k c", c=c)
                    wb = wgt.to_broadcast([P, TOPK, c])

                    if use_dma_gather:
                        # subtile-local row index = p*64 + expert (int16-safe)
                        idx16 = spool.tile([P, 8], i16, tag="idx16", name="idx16")
                        nc.vector.tensor_tensor(
                            out=idx16, in0=ti,
                            in1=cst["rb64"].to_broadcast([P, 8]), op=OP.add)
                        # wrap to dma_gather's idx layout via a DRAM bounce:
                        # flat gather position i = slot*128 + p lives at
                        # [i%16, i//16], and the [16, 64] block must be
                        # REPLICATED to all 8 partition groups (each Q7 cpu
                        # pair reads its own 16-partition stripe).
                        # idxD pi-major: idxD[pi*64 + j*8 + po] = idx(p,j),
                        # p = po*16 + pi  -> block row pi, col s = j*8+po
                        idxD = dpool.tile([TOPK * P], i16, tag="idxD",
                                          name="idxD")
                        nc.scalar.dma_start(
                            out=idxD.rearrange("(pi j po) -> po pi j",
                                               pi=16, j=TOPK),
                            in_=idx16)
                        idxs_sb = ipool.tile([P, TOPK * P // 16], i16,
                                             tag="idxs", name="idxs")
                        nc.scalar.dma_start(
                            out=idxs_sb,
                            in_=idxD.rearrange("(pi s) -> pi s", pi=16)[None]
                            .broadcast_to([8, 16, TOPK * P // 16]))
                        nc.gpsimd.dma_gather(
                            sel, pN[bg * P * e:(bg + 1) * P * e, :], idxs_sb,
                            TOPK * P, TOPK * P, c)
                        nc.vector.tensor_tensor(out=mt3, in0=sel, in1=wb,
                                                op=OP.mult)
                        for q in range(4):
                            nc.vector.tensor_add(
                                mt[:, 2 * q * c:(2 * q + 1) * c],
                                mt[:, 2 * q * c:(2 * q + 1) * c],
                                mt[:, (2 * q + 1) * c:(2 * q + 2) * c])
                    else:
                        # DRAM row index = (bg*128 + p)*e + expert
                        ridx = spool.tile([P, 8], u32, tag="ridx", name="ridx")
                        nc.vector.tensor_tensor(
                            out=ridx, in0=ti,
                            in1=cst[f"rb_{bg}"].to_broadcast([P, 8]), op=OP.add)
                        for q in range(4):
                            ks = slice(2 * q, 2 * q + 2)
                            for kk in range(2 * q, 2 * q + 2):
                                nc.gpsimd.indirect_dma_start(
                                    out=sel[:, kk, :], out_offset=None, in_=pN,
                                    in_offset=bass.IndirectOffsetOnAxis(
                                        ap=ridx[:, kk:kk + 1], axis=0))
                            nc.vector.tensor_tensor(out=mt3[:, ks, :],
                                                    in0=sel[:, ks, :],
                                                    in1=wb[:, ks, :], op=OP.mult)
                            nc.vector.tensor_add(
                                mt[:, 2 * q * c:(2 * q + 1) * c],
                                mt[:, 2 * q * c:(2 * q + 1) * c],
                                mt[:, (2 * q + 1) * c:(2 * q + 2) * c])
                    nc.vector.tensor_add(mt[:, :c], mt[:, :c], mt[:, 2 * c:3 * c])
                    nc.vector.tensor_add(mt[:, 4 * c:5 * c], mt[:, 4 * c:5 * c],
                                         mt[:, 6 * c:7 * c])
                    acc = opool.tile([P, c], f16, tag=f"acc_{bg}", name=f"acc_{bg}")
                    nc.vector.tensor_add(acc, mt[:, :c], mt[:, 4 * c:5 * c])
                    acc_tiles.append((bg, acc))

                    if debug_taps:
                        rows = slice(bg * P, (bg + 1) * P)
                        lgc = spool.tile([P, e], f32, tag="lgc", name="lgc")
                        nc.vector.tensor_copy(out=lgc, in_=ps_lg)
                        nc.sync.dma_start(out=dbg_lg[rows, :], in_=lgc)
                        nc.sync.dma_start(out=dbg_ti[rows, :], in_=ti)
                        nc.sync.dma_start(out=dbg_w[rows, :], in_=wgt)

            nc.__enter_lp = nc.allow_low_precision(
                reason="fp16 weighted-sum tree of fp16 gathers; output "
                       "stores are fp16 regardless")
            nc.__enter_lp.__enter__()

            col0 = 0
            pending = None
            for n, nt in enumerate(tiles):
                toff = KC * col0  # column offset of this tile's block in pTr
                ps_h1 = [psh1.tile([P, nt], f32, tag=f"h1_{m}", name=f"ps_h1_{m}")
                         for m in range(MC)]
                for g in range(KG):
                    xt = xpool.tile([P, kper, nt], f16, tag="xt", name="xt")
                    if n == 0 and g == 0:
                        # per-chunk DMAs so the first matmul only waits for
                        # chunk 0 of xt and W1, not the whole group
                        wt = w1pool.tile([P, kper, h1], f16, tag="w1_0",
                                         name="w1_0")
                        w1_tiles[0] = wt
                        for cc in range(kper):
                            nc.sync.dma_start(
                                out=xt[:, cc, :],
                                in_=pTr[:, toff + cc * nt:toff + (cc + 1) * nt])
                            nc.sync.dma_start(
                                out=wt[:, cc, :],
                                in_=w1r[:, cc * h1:(cc + 1) * h1])
                    else:
                        nc.sync.dma_start(
                            out=xt,
                            in_=pTr[:, toff + g * kper * nt:
                                    toff + (g + 1) * kper * nt]
                            .rearrange("p (c b) -> p c b", c=kper))
                        if n == 0:
                            wt = w1pool.tile([P, kper, h1], f16, tag=f"w1_{g}",
                                             name=f"w1_{g}")
                            nc.sync.dma_start(
                                out=wt,
                                in_=w1r[:, g * kper * h1:(g + 1) * kper * h1]
                                .rearrange("p (c h) -> p c h", c=kper))
                            w1_tiles[g] = wt
                    wt = w1_tiles[g]
                    for cc in range(kper):
                        for m in range(MC):
                            nc.tensor.matmul(
                                out=ps_h1[m], lhsT=wt[:, cc, m * P:(m + 1) * P],
                                rhs=xt[:, cc, :],
                                start=(g == 0 and cc == 0),
                                stop=(g == KG - 1 and cc == kper - 1))
                    if n == 0 and g == 2:
                        emit_consts()
                    if g == 1 and pending is not None:
                        # previous tile's stage-2/3 + routing, enqueued only
                        # now so the PE queue never stalls on the relu at the
                        # tile boundary
                        finish_tile(*pending)
                        pending = None

                pending = (nt, col0, ps_h1)
                col0 += nt

            finish_tile(*pending)

            nc.__enter_lp.__exit__(None, None, None)

            # output stores, all at the tail of the in-order sync queue: by
            # the time the queue drains the xt stream, the early acc tiles
            # are long done
            for bg, acc in acc_tiles:
                nc.sync.dma_start(out=out[bg * P:(bg + 1) * P, :], in_=acc)

    nc.compile()
    return nc


def _prep_core_inputs(flat, W1b, w1r_, w2r_, w3_, b1, b2, b3, core,
                      tiles=TILES, kper=KPER):
    KC = D // P
    shard = flat[core * BS:(core + 1) * BS]                    # (BS, D)
    hf = shard.astype(np.float16)                              # (BS, D)
    blocks = []
    col0 = 0
    for nt in tiles:
        blk = hf[col0:col0 + nt, :].T                          # (D, nt)
        blocks.append(np.ascontiguousarray(
            blk.reshape(KC, P, nt).transpose(1, 0, 2).reshape(P, KC * nt)))
        col0 += nt
    pTr = np.concatenate(blocks, axis=1)                       # (P, KC*BS)
    pN = np.ascontiguousarray(hf).reshape(BS * E, C)
    b1r = np.ascontiguousarray(b1.reshape(H1 // P, P).T)
    b2r = np.ascontiguousarray(b2.reshape(H2, 1))
    b3r = np.ascontiguousarray(np.broadcast_to(b3, (P, E)))
    return {"pTr": pTr, "pN": pN, "w1r": w1r_, "w2r": w2r_, "w3": w3_,
            "b1r": b1r, "b2r": b2r, "b3r": b3r}


def _prep_weights(W1, W2, W3):
    KC = D // P
    w1r_ = np.ascontiguousarray(
        W1.astype(np.float16).reshape(KC, P, H1)
        .transpose(1, 0, 2).reshape(P, KC * H1))
    w2r_ = np.ascontiguousarray(
        W2.reshape(H1 // P, P, H2).transpose(1, 0, 2).reshape(P, -1))
    w3_ = np.ascontiguousarray(W3)
    return w1r_, w2r_, w3_


def kernel(expert_probs, W1, b1, W2, b2, W3, b3):
    from concourse.bass_utils import run_bass_kernel_spmd

    expert_probs = np.asarray(expert_probs, dtype=np.float32)
    W1 = np.asarray(W1, dtype=np.float32)
    W2 = np.asarray(W2, dtype=np.float32)
    W3 = np.asarray(W3, dtype=np.float32)
    b1 = np.asarray(b1, dtype=np.float32)
    b2 = np.asarray(b2, dtype=np.float32)
    b3 = np.asarray(b3, dtype=np.float32)

    if "nc" not in _BUILD_CACHE:
        _BUILD_CACHE["nc"] = build_moe_nc()
    nc = _BUILD_CACHE["nc"]

    flat = expert_probs.reshape(B, D)
    w1r_, w2r_, w3_ = _prep_weights(W1, W2, W3)
    in_maps = [_prep_core_inputs(flat, None, w1r_, w2r_, w3_, b1, b2, b3, cid)
               for cid in range(NCORES)]
    res = run_bass_kernel_spmd(nc, in_maps, core_ids=list(range(NCORES)))
    out = np.concatenate([res.results[cid]["out"] for cid in range(NCORES)], axis=0)
    return out.astype(np.float32)  # device stores fp16; ~1e-4 rel quantization


# revision 6
# speedup vs baseline: 1.0393x; 1.0393x over previous
"""MoE routing kernel for Trainium2 (8 NeuronCores, data-parallel over batch).

Stage-1 matmul in fp16 (4x PE rate vs fp32, half the HBM bytes
for the dominant x^T stream, pre-converted host-side); gather stream
(pN) and top-8 mixing weights also fp16. Softmax/top-8 selection stays
f32. 8-chunk DMA groups give 2-4KB contiguous runs per partition.
Tile mix (128,256,256,256,128) spreads the W1 load crunch.

Gather: per 128-row subtile, TWO dma_gather calls (512 idxs each, 4
selected expert rows per batch row) on rotating SWDGE queues (4 Q7
cpu pairs run desc-gen concurrently; the Q7 idx-unpack loop costs
~8.4ns/idx so a 1024-idx gather is ~8.6us serial — split halves
run ~4.4us in parallel). dma_gather wants int16 row indices wrapped
16-partition-style (flat gather position i = slot*128 + p lives at
[i%16, i//16]) and REPLICATED to all 8 partition groups (each Q7 cpu
pair reads its own 16-partition stripe; zeros there = it gathers row
0). Indices are subtile-local (p*64+e < 8192, int16-safe; gather base
= pN + bg*8192 rows). The wrap+replicate runs through a DRAM bounce
(SBUF [128,8] -> pi-major DRAM block -> broadcast-read back to
[128, 64]); a half-gather's idx block is just columns [32h, 32h+32)
of the full block.

Scheduling: the routing for tile t is emitted in three phases against
tile t+1's k-group stream: phase1 (relu/stage-2/3, softmax, top-8,
idx compute, bounce leg A) at g==1, phase2 (bounce leg B) at g==3,
phase3 (gathers + weighted sum) at g==5. Both bounce legs ride the
sync queue separated by an x-group DMA, so leg B never waits at the
queue head (leg A's completion is long past), the scalar queue stays
pure relu/exp (an earlier revision put the bounce there and the
B-leg's completion wait blocked the next tile's relu -> PSUM never
recycled -> PE+DMA death spiral at 2x the baseline time), and the
gpsimd queue only carries the gathers themselves.

Pipeline per core (batch shard of 1024 rows):
  h1^T = relu(W1^T @ flat^T + b1)   # contraction D=16384, fp16 on PE
  h2^T = relu(W2^T @ h1^T + b2)
  logits = h2^T.T @ W3 + b3         # [128b, 64e] tiles
  s = softmax(logits) in f32; top-8 via DVE max/max_index;
  weights = top_vals / sum(top_vals)
  dma_gather selected expert rows; weighted sum; store.

Host-side layout: pTr[p, (t,g,cc,b)] so each (tile, k-group) DMA is a
single [128 x kper*nt] straight copy with kper*nt*2B contiguous runs
per partition (4 KB for the 256-wide tiles).
"""

import numpy as np

B, E, C, TOPK = 8192, 64, 256, 8
D, H1, H2 = 16384, 256, 128
NCORES = 8
BS = B // NCORES  # batch rows per core
P = 128
KPER = 8
TILES = (128, 256, 256, 256, 128)
NQ = 4  # SWDGE queues

_BUILD_CACHE = {}


def build_moe_nc(bs=BS, d=D, e=E, c=C, h1=H1, h2=H2, tiles=TILES, kper=KPER,
                 xbufs=12, hbufs=3, debug_taps=False):
    import concourse.bacc as bacc
    import concourse.bass as bass
    import concourse.mybir as mybir
    from concourse import tile

    f32 = mybir.dt.float32
    f16 = mybir.dt.float16
    u32 = mybir.dt.uint32
    i16 = mybir.dt.int16
    KC = d // P            # 128-row K-chunks in main contraction
    KG = KC // kper        # DMA groups of kper chunks
    MC = h1 // P           # output row chunks of h1^T
    assert sum(tiles) == bs
    KC2 = h1 // P          # K-chunks for stage 2
    NBT = bs // P          # total 128-row batch subtiles
    SWRAP = TOPK * P // 16  # 64 wrapped idx columns per subtile

    nc = bacc.Bacc("TRN2", target_bir_lowering=False, debug=False,
                   num_devices=NCORES, num_swdge_queues=NQ)

    pTr = nc.dram_tensor("pTr", [P, KC * bs], f16, kind="ExternalInput").ap()
    pN = nc.dram_tensor("pN", [bs * e, c], f16, kind="ExternalInput").ap()
    w1r = nc.dram_tensor("w1r", [P, KC * h1], f16, kind="ExternalInput").ap()
    w2r = nc.dram_tensor("w2r", [P, KC2 * h2], f32, kind="ExternalInput").ap()
    w3 = nc.dram_tensor("w3", [h2, e], f32, kind="ExternalInput").ap()
    b1r = nc.dram_tensor("b1r", [P, MC], f32, kind="ExternalInput").ap()
    b2r = nc.dram_tensor("b2r", [P, 1], f32, kind="ExternalInput").ap()
    b3r = nc.dram_tensor("b3r", [P, e], f32, kind="ExternalInput").ap()
    out = nc.dram_tensor("out", [bs, c], f16, kind="ExternalOutput").ap()

    AF = mybir.ActivationFunctionType
    OP = mybir.AluOpType

    with tile.TileContext(nc) as tc:
        with (
            tc.tile_pool(name="wconst", bufs=1) as wconst,
            tc.tile_pool(name="w1pool", bufs=1) as w1pool,
            tc.tile_pool(name="xpool", bufs=xbufs) as xpool,
            tc.tile_pool(name="hpool", bufs=hbufs) as hpool,
            tc.tile_pool(name="spool", bufs=6) as spool,
            tc.tile_pool(name="selpool", bufs=4) as selpool,
            tc.tile_pool(name="ipool", bufs=3) as ipool,
            tc.tile_pool(name="dpool", bufs=3, space="DRAM") as dpool,
            tc.tile_pool(name="opool", bufs=1) as opool,
            tc.tile_pool(name="psh1", bufs=2, space="PSUM") as psh1,
            tc.tile_pool(name="psh2", bufs=2, space="PSUM") as psh2,
            tc.tile_pool(name="pslg", bufs=2, space="PSUM") as pslg,
        ):
            # --- constants (small); emitted AFTER the first k-group DMAs so
            # they don't delay the PE-critical xt/W1 stream at startup
            cst = {}

            def emit_consts():
                cst["w2"] = wconst.tile([P, KC2 * h2], f32, name="w2_sb")
                nc.scalar.dma_start(out=cst["w2"], in_=w2r)
                cst["w3"] = wconst.tile([P, e], f32, name="w3_sb")
                nc.scalar.dma_start(out=cst["w3"][:h2, :], in_=w3)
                cst["b1"] = wconst.tile([P, MC], f32, name="b1_sb")
                nc.scalar.dma_start(out=cst["b1"], in_=b1r)
                cst["b2"] = wconst.tile([P, 1], f32, name="b2_sb")
                nc.scalar.dma_start(out=cst["b2"], in_=b2r)
                cst["b3"] = wconst.tile([P, e], f32, name="b3_sb")
                nc.scalar.dma_start(out=cst["b3"], in_=b3r)
                # rb64[p] = p*64: subtile-local DRAM row base per partition
                rb = wconst.tile([P, 1], u32, name="rb64")
                nc.gpsimd.iota(rb, pattern=[[0, 1]], base=0,
                               channel_multiplier=e)
                cst["rb64"] = rb

            # --- W1 group tiles: persistent, loaded just-in-time in n=0 loop
            w1_tiles = [None] * KG
            acc_tiles = []

            def routing_p1(nt, col0, ps_h1):
                """relu/stage-2/3 + softmax/top-8 + idx compute + bounce leg A.
                Returns per-subtile state for p2/p3."""
                h1r = []
                for m in range(MC):
                    hr = hpool.tile([P, nt], f32, tag=f"h1r_{m}", name=f"h1r_{m}")
                    nc.scalar.activation(hr, ps_h1[m], AF.Relu,
                                         bias=cst["b1"][:, m:m + 1])
                    h1r.append(hr)

                ps_h2 = psh2.tile([P, nt], f32, tag="h2", name="ps_h2")
                for k2 in range(KC2):
                    nc.tensor.matmul(out=ps_h2[:h2, :],
                                     lhsT=cst["w2"][:, k2 * h2:(k2 + 1) * h2],
                                     rhs=h1r[k2], start=(k2 == 0),
                                     stop=(k2 == KC2 - 1))
                h2r = hpool.tile([P, nt], f32, tag="h2r", name="h2r")
                nc.scalar.activation(h2r[:h2, :], ps_h2[:h2, :], AF.Relu,
                                     bias=cst["b2"][:h2, :])

                state = []
                for bt in range(nt // P):
                    bg = col0 // P + bt  # global 128-row batch subtile index
                    ps_lg = pslg.tile([P, e], f32, tag="lg", name="ps_lg")
                    nc.tensor.matmul(out=ps_lg, lhsT=h2r[:h2, bt * P:(bt + 1) * P],
                                     rhs=cst["w3"][:h2, :], start=True, stop=True)
                    lg = spool.tile([P, e], f32, tag="lg_sb", name="lg_sb")
                    nc.vector.tensor_tensor(out=lg, in0=ps_lg, in1=cst["b3"], op=OP.add)

                    # f32 softmax, replicating the reference's quantization
                    nm = spool.tile([P, 1], f32, tag="nm", name="nm")
                    nc.vector.reduce_max(out=nm, in_=lg, axis=mybir.AxisListType.X,
                                         negate=True)
                    ef = spool.tile([P, e], f32, tag="ef", name="ef")
                    nc.scalar.activation(ef, lg, AF.Exp, bias=nm)
                    # top-8 straight on the unnormalized exps: selection order
                    # is scale-invariant and tv/sum(tv) cancels the softmax
                    # normalization
                    tv = spool.tile([P, 8], f32, tag="tv", name="tv")
                    nc.vector.max(out=tv, in_=ef)
                    ti = spool.tile([P, 8], u32, tag="ti", name="ti")
                    nc.vector.max_index(out=ti, in_max=tv, in_values=ef)

                    s8 = spool.tile([P, 1], f32, tag="s8", name="s8")
                    nc.vector.reduce_sum(out=s8, in_=tv, axis=mybir.AxisListType.X)
                    r8 = spool.tile([P, 1], f32, tag="r8", name="r8")
                    nc.vector.reciprocal(r8, s8)
                    wgt = spool.tile([P, 8], f16, tag="wgt", name="wgt")
                    nc.scalar.activation(wgt, tv, AF.Copy, scale=r8)

                    # subtile-local row index = p*64 + expert (int16-safe)
                    idx16 = spool.tile([P, 8], i16, tag="idx16", name="idx16")
                    nc.vector.tensor_tensor(
                        out=idx16, in0=ti,
                        in1=cst["rb64"].to_broadcast([P, 8]), op=OP.add)
                    # bounce leg A: pi-major wrapped block in DRAM:
                    # idxD[pi*64 + j*8 + po] = idx(p=po*16+pi, slot j)
                    idxD = dpool.tile([TOPK * P], i16, tag="idxD", name="idxD")
                    nc.sync.dma_start(
                        out=idxD.rearrange("(pi j po) -> po pi j",
                                           pi=16, j=TOPK),
                        in_=idx16)
                    state.append({"bg": bg, "wgt": wgt, "idxD": idxD})
                return state

            def routing_p2(state):
                """bounce leg B: broadcast-read the wrapped block into all 8
                partition groups (each Q7 cpu pair reads its own stripe)."""
                for st in state:
                    idxs_sb = ipool.tile([P, SWRAP], i16, tag="idxs",
                                         name="idxs")
                    nc.sync.dma_start(
                        out=idxs_sb,
                        in_=st["idxD"].rearrange("(pi s) -> pi s", pi=16)[None]
                        .broadcast_to([8, 16, SWRAP]))
                    st["idxs"] = idxs_sb

            def routing_p3(state):
                """half-gathers on rotating SWDGE queues + weighted sum."""
                for st in state:
                    bg, wgt, idxs_sb = st["bg"], st["wgt"], st["idxs"]
                    sel = selpool.tile([P, TOPK, c], f16, tag="sel", name="sel")
                    mt = selpool.tile([P, TOPK * c], f16, tag="mt", name="mt")
                    mt3 = mt.rearrange("p (k c) -> p k c", c=c)
                    wb = wgt.to_broadcast([P, TOPK, c])
                    pNsub = pN[bg * P * e:(bg + 1) * P * e, :]
                    for h in range(2):
                        nc.gpsimd.dma_gather(
                            sel[:, 4 * h:4 * h + 4, :], pNsub,
                            idxs_sb[:, 32 * h:32 * (h + 1)],
                            TOPK * P // 2, TOPK * P // 2, c,
                            queue_num=(2 * bg + h) % NQ)
                    for h in range(2):
                        nc.vector.tensor_tensor(
                            out=mt3[:, 4 * h:4 * h + 4, :],
                            in0=sel[:, 4 * h:4 * h + 4, :],
                            in1=wb[:, 4 * h:4 * h + 4, :], op=OP.mult)
                    for q in range(4):
                        nc.vector.tensor_add(
                            mt[:, 2 * q * c:(2 * q + 1) * c],
                            mt[:, 2 * q * c:(2 * q + 1) * c],
                            mt[:, (2 * q + 1) * c:(2 * q + 2) * c])
                    nc.vector.tensor_add(mt[:, :c], mt[:, :c], mt[:, 2 * c:3 * c])
                    nc.vector.tensor_add(mt[:, 4 * c:5 * c], mt[:, 4 * c:5 * c],
                                         mt[:, 6 * c:7 * c])
                    acc = opool.tile([P, c], f16, tag=f"acc_{bg}",
                                     name=f"acc_{bg}")
                    nc.vector.tensor_add(acc, mt[:, :c], mt[:, 4 * c:5 * c])
                    acc_tiles.append((bg, acc))

            nc.__enter_lp = nc.allow_low_precision(
                reason="fp16 weighted-sum tree of fp16 gathers; output "
                       "stores are fp16 regardless")
            nc.__enter_lp.__enter__()

            col0 = 0
            pending = None
            p1state = None
            for n, nt in enumerate(tiles):
                toff = KC * col0  # column offset of this tile's block in pTr
                ps_h1 = [psh1.tile([P, nt], f32, tag=f"h1_{m}", name=f"ps_h1_{m}")
                         for m in range(MC)]
                for g in range(KG):
                    xt = xpool.tile([P, kper, nt], f16, tag="xt", name="xt")
                    if n == 0 and g == 0:
                        # per-chunk DMAs so the first matmul only waits for
                        # chunk 0 of xt and W1, not the whole group
                        wt = w1pool.tile([P, kper, h1], f16, tag="w1_0",
                                         name="w1_0")
                        w1_tiles[0] = wt
                        for cc in range(kper):
                            nc.sync.dma_start(
                                out=xt[:, cc, :],
                                in_=pTr[:, toff + cc * nt:toff + (cc + 1) * nt])
                            nc.sync.dma_start(
                                out=wt[:, cc, :],
                                in_=w1r[:, cc * h1:(cc + 1) * h1])
                    else:
                        nc.sync.dma_start(
                            out=xt,
                            in_=pTr[:, toff + g * kper * nt:
                                    toff + (g + 1) * kper * nt]
                            .rearrange("p (c b) -> p c b", c=kper))
                        if n == 0:
                            wt = w1pool.tile([P, kper, h1], f16, tag=f"w1_{g}",
                                             name=f"w1_{g}")
                            nc.sync.dma_start(
                                out=wt,
                                in_=w1r[:, g * kper * h1:(g + 1) * kper * h1]
                                .rearrange("p (c h) -> p c h", c=kper))
                            w1_tiles[g] = wt
                    wt = w1_tiles[g]
                    for cc in range(kper):
                        for m in range(MC):
                            nc.tensor.matmul(
                                out=ps_h1[m], lhsT=wt[:, cc, m * P:(m + 1) * P],
                                rhs=xt[:, cc, :],
                                start=(g == 0 and cc == 0),
                                stop=(g == KG - 1 and cc == kper - 1))
                    if n == 0 and g == 2:
                        emit_consts()
                    if pending is not None:
                        # previous tile's routing, spread across this tile's
                        # k-group stream (phases separated by x-group DMAs on
                        # the sync queue so no queue waits at its head)
                        if g == 1:
                            p1state = routing_p1(*pending)
                        elif g == 3:
                            routing_p2(p1state)
                        elif g == 5:
                            routing_p3(p1state)
                            pending = None
                            p1state = None

                pending = (nt, col0, ps_h1)
                col0 += nt

            p1state = routing_p1(*pending)
            routing_p2(p1state)
            routing_p3(p1state)

            nc.__enter_lp.__exit__(None, None, None)

            # output stores, all at the tail of the in-order sync queue: by
            # the time the queue drains the xt stream, the early acc tiles
            # are long done
            for bg, acc in acc_tiles:
                nc.sync.dma_start(out=out[bg * P:(bg + 1) * P, :], in_=acc)

    nc.compile()
    return nc


def _prep_core_inputs(flat, W1b, w1r_, w2r_, w3_, b1, b2, b3, core,
                      tiles=TILES, kper=KPER):
    KC = D // P
    shard = flat[core * BS:(core + 1) * BS]                    # (BS, D)
    hf = shard.astype(np.float16)                              # (BS, D)
    blocks = []
    col0 = 0
    for nt in tiles:
        blk = hf[col0:col0 + nt, :].T                          # (D, nt)
        blocks.append(np.ascontiguousarray(
            blk.reshape(KC, P, nt).transpose(1, 0, 2).reshape(P, KC * nt)))
        col0 += nt
    pTr = np.concatenate(blocks, axis=1)                       # (P, KC*BS)
    pN = np.ascontiguousarray(hf).reshape(BS * E, C)
    b1r = np.ascontiguousarray(b1.reshape(H1 // P, P).T)
    b2r = np.ascontiguousarray(b2.reshape(H2, 1))
    b3r = np.ascontiguousarray(np.broadcast_to(b3, (P, E)))
    return {"pTr": pTr, "pN": pN, "w1r": w1r_, "w2r": w2r_, "w3": w3_,
            "b1r": b1r, "b2r": b2r, "b3r": b3r}


def _prep_weights(W1, W2, W3):
    KC = D // P
    w1r_ = np.ascontiguousarray(
        W1.astype(np.float16).reshape(KC, P, H1)
        .transpose(1, 0, 2).reshape(P, KC * H1))
    w2r_ = np.ascontiguousarray(
        W2.reshape(H1 // P, P, H2).transpose(1, 0, 2).reshape(P, -1))
    w3_ = np.ascontiguousarray(W3)
    return w1r_, w2r_, w3_


def kernel(expert_probs, W1, b1, W2, b2, W3, b3):
    from concourse.bass_utils import run_bass_kernel_spmd

    expert_probs = np.asarray(expert_probs, dtype=np.float32)
    W1 = np.asarray(W1, dtype=np.float32)
    W2 = np.asarray(W2, dtype=np.float32)
    W3 = np.asarray(W3, dtype=np.float32)
    b1 = np.asarray(b1, dtype=np.float32)
    b2 = np.asarray(b2, dtype=np.float32)
    b3 = np.asarray(b3, dtype=np.float32)

    if "nc" not in _BUILD_CACHE:
        _BUILD_CACHE["nc"] = build_moe_nc()
    nc = _BUILD_CACHE["nc"]

    flat = expert_probs.reshape(B, D)
    w1r_, w2r_, w3_ = _prep_weights(W1, W2, W3)
    in_maps = [_prep_core_inputs(flat, None, w1r_, w2r_, w3_, b1, b2, b3, cid)
               for cid in range(NCORES)]
    res = run_bass_kernel_spmd(nc, in_maps, core_ids=list(range(NCORES)))
    out = np.concatenate([res.results[cid]["out"] for cid in range(NCORES)], axis=0)
    return out.astype(np.float32)  # device stores fp16; ~1e-4 rel quantization


# revision 11
# speedup vs baseline: 1.6971x; 1.6329x over previous
"""MoE routing kernel for Trainium2 (8 NeuronCores, data-parallel over batch).

Stage-1 matmul in fp16 (4x PE rate vs fp32, half the HBM bytes
for the dominant x^T stream, pre-converted host-side); gather stream
(pN) and top-8 mixing weights also fp16. Softmax/top-8 selection stays
f32. 8-chunk DMA groups give 2-4KB contiguous runs per partition.
Tile mix (128,256,256,256,128) spreads the W1 load crunch.

Gather: per 128-row subtile, TWO dma_gather calls (512 idxs each, 4
selected expert rows per batch row) on rotating SWDGE queues (4 Q7
cpu pairs run desc-gen concurrently; the Q7 idx-unpack loop costs
~8.4ns/idx so a 1024-idx gather is ~8.6us serial — split halves
run ~4.4us in parallel). dma_gather wants int16 row indices wrapped
16-partition-style (flat gather position i = slot*128 + p lives at
[i%16, i//16]) and REPLICATED to all 8 partition groups (each Q7 cpu
pair reads its own 16-partition stripe; zeros there = it gathers row
0). Indices are subtile-local (p*64+e < 8192, int16-safe; gather base
= pN + bg*8192 rows). The wrap+replicate runs through a DRAM bounce
(SBUF [128,8] -> pi-major DRAM block -> broadcast-read back to
[128, 64]); a half-gather's idx block is just columns [32h, 32h+32)
of the full block.

Scheduling: the routing for tile t is emitted in three phases against
tile t+1's k-group stream: phase1 (relu/stage-2/3, softmax, top-8,
idx compute, bounce leg A) at g==1, phase2 (bounce leg B) at g==3,
phase3 (gathers + weighted sum) at g==5. Both bounce legs ride the
sync queue separated by an x-group DMA, so leg B never waits at the
queue head (leg A's completion is long past), the scalar queue stays
pure relu/exp (an earlier revision put the bounce there and the
B-leg's completion wait blocked the next tile's relu -> PSUM never
recycled -> PE+DMA death spiral at 2x the baseline time), and the
gpsimd queue only carries the gathers themselves.

Pipeline per core (batch shard of 1024 rows):
  h1^T = relu(W1^T @ flat^T + b1)   # contraction D=16384, fp16 on PE
  h2^T = relu(W2^T @ h1^T + b2)
  logits = h2^T.T @ W3 + b3         # [128b, 64e] tiles
  s = softmax(logits) in f32; top-8 via DVE max/max_index;
  weights = top_vals / sum(top_vals)
  dma_gather selected expert rows; weighted sum; store.

Host-side layout: pTr[p, (t,g,cc,b)] so each (tile, k-group) DMA is a
single [128 x kper*nt] straight copy with kper*nt*2B contiguous runs
per partition (4 KB for the 256-wide tiles).
"""

import numpy as np

B, E, C, TOPK = 8192, 64, 256, 8
D, H1, H2 = 16384, 256, 128
NCORES = 8
BS = B // NCORES  # batch rows per core
P = 128
KPER = 8
TILES = (128, 256, 256, 256, 128)
NQ = 4  # SWDGE queues

_BUILD_CACHE = {}


def build_moe_nc(bs=BS, d=D, e=E, c=C, h1=H1, h2=H2, tiles=TILES, kper=KPER,
                 xbufs=12, hbufs=3, debug_taps=False):
    import concourse.bacc as bacc
    import concourse.bass as bass
    import concourse.mybir as mybir
    from concourse import tile

    f32 = mybir.dt.float32
    f16 = mybir.dt.float16
    u32 = mybir.dt.uint32
    i16 = mybir.dt.int16
    KC = d // P            # 128-row K-chunks in main contraction
    KG = KC // kper        # DMA groups of kper chunks
    MC = h1 // P           # output row chunks of h1^T
    assert sum(tiles) == bs
    KC2 = h1 // P          # K-chunks for stage 2
    NBT = bs // P          # total 128-row batch subtiles
    SWRAP = TOPK * P // 16  # 64 wrapped idx columns per subtile

    nc = bacc.Bacc("TRN2", target_bir_lowering=False, debug=False,
                   num_devices=NCORES, num_swdge_queues=NQ)

    pTr = nc.dram_tensor("pTr", [P, KC * bs], f16, kind="ExternalInput").ap()
    pN = nc.dram_tensor("pN", [bs * e, c], f16, kind="ExternalInput").ap()
    w1r = nc.dram_tensor("w1r", [P, KC * h1], f16, kind="ExternalInput").ap()
    w2r = nc.dram_tensor("w2r", [P, KC2 * h2], f32, kind="ExternalInput").ap()
    w3 = nc.dram_tensor("w3", [h2, e], f32, kind="ExternalInput").ap()
    b1r = nc.dram_tensor("b1r", [P, MC], f32, kind="ExternalInput").ap()
    b2r = nc.dram_tensor("b2r", [P, 1], f32, kind="ExternalInput").ap()
    b3r = nc.dram_tensor("b3r", [P, e], f32, kind="ExternalInput").ap()
    out = nc.dram_tensor("out", [bs, c], f16, kind="ExternalOutput").ap()

    AF = mybir.ActivationFunctionType
    OP = mybir.AluOpType

    with tile.TileContext(nc) as tc:
        with (
            tc.tile_pool(name="wconst", bufs=1) as wconst,
            tc.tile_pool(name="w1pool", bufs=1) as w1pool,
            tc.tile_pool(name="xpool", bufs=xbufs) as xpool,
            tc.tile_pool(name="hpool", bufs=hbufs) as hpool,
            tc.tile_pool(name="spool", bufs=6) as spool,
            tc.tile_pool(name="selpool", bufs=4) as selpool,
            tc.tile_pool(name="ipool", bufs=3) as ipool,
            tc.tile_pool(name="dpool", bufs=3, space="DRAM") as dpool,
            tc.tile_pool(name="opool", bufs=1) as opool,
            tc.tile_pool(name="psh1", bufs=2, space="PSUM") as psh1,
            tc.tile_pool(name="psh2", bufs=2, space="PSUM") as psh2,
            tc.tile_pool(name="pslg", bufs=2, space="PSUM") as pslg,
        ):
            # --- constants (small); emitted AFTER the first k-group DMAs so
            # they don't delay the PE-critical xt/W1 stream at startup
            cst = {}

            def emit_consts():
                cst["w2"] = wconst.tile([P, KC2 * h2], f32, name="w2_sb")
                nc.scalar.dma_start(out=cst["w2"], in_=w2r)
                cst["w3"] = wconst.tile([P, e], f32, name="w3_sb")
                nc.scalar.dma_start(out=cst["w3"][:h2, :], in_=w3)
                cst["b1"] = wconst.tile([P, MC], f32, name="b1_sb")
                nc.scalar.dma_start(out=cst["b1"], in_=b1r)
                cst["b2"] = wconst.tile([P, 1], f32, name="b2_sb")
                nc.scalar.dma_start(out=cst["b2"], in_=b2r)
                cst["b3"] = wconst.tile([P, e], f32, name="b3_sb")
                nc.scalar.dma_start(out=cst["b3"], in_=b3r)
                # rb64[p] = p*64: subtile-local DRAM row base per partition
                rb = wconst.tile([P, 1], u32, name="rb64")
                nc.gpsimd.iota(rb, pattern=[[0, 1]], base=0,
                               channel_multiplier=e)
                cst["rb64"] = rb
                # idxs tiles: only partitions 0-63 (queue-0/1 Q7 windows) are
                # ever written; zero the rest once so every partition holds a
                # valid in-bounds index (the sim checks all 128)
                for t in range(3):
                    it = ipool.tile([P, SWRAP], i16, tag=f"idxs_{t}",
                                    name=f"idxs_{t}")
                    nc.vector.memset(it, 0)
                    cst[f"idxs_{t}"] = it

            # --- W1 group tiles: persistent, loaded just-in-time in n=0 loop
            w1_tiles = [None] * KG
            acc_tiles = []

            def routing_p1(nt, col0, ps_h1):
                """relu/stage-2/3 + softmax/top-8 + idx compute + bounce leg A.
                Returns per-subtile state for p2/p3."""
                h1r = []
                for m in range(MC):
                    hr = hpool.tile([P, nt], f32, tag=f"h1r_{m}", name=f"h1r_{m}")
                    nc.scalar.activation(hr, ps_h1[m], AF.Relu,
                                         bias=cst["b1"][:, m:m + 1])
                    h1r.append(hr)

                ps_h2 = psh2.tile([P, nt], f32, tag="h2", name="ps_h2")
                for k2 in range(KC2):
                    nc.tensor.matmul(out=ps_h2[:h2, :],
                                     lhsT=cst["w2"][:, k2 * h2:(k2 + 1) * h2],
                                     rhs=h1r[k2], start=(k2 == 0),
                                     stop=(k2 == KC2 - 1))
                h2r = hpool.tile([P, nt], f32, tag="h2r", name="h2r")
                nc.scalar.activation(h2r[:h2, :], ps_h2[:h2, :], AF.Relu,
                                     bias=cst["b2"][:h2, :])

                state = []
                for bt in range(nt // P):
                    bg = col0 // P + bt  # global 128-row batch subtile index
                    ps_lg = pslg.tile([P, e], f32, tag="lg", name="ps_lg")
                    nc.tensor.matmul(out=ps_lg, lhsT=h2r[:h2, bt * P:(bt + 1) * P],
                                     rhs=cst["w3"][:h2, :], start=True, stop=True)
                    lg = spool.tile([P, e], f32, tag="lg_sb", name="lg_sb")
                    nc.vector.tensor_tensor(out=lg, in0=ps_lg, in1=cst["b3"], op=OP.add)

                    # f32 softmax, replicating the reference's quantization
                    nm = spool.tile([P, 1], f32, tag="nm", name="nm")
                    nc.vector.reduce_max(out=nm, in_=lg, axis=mybir.AxisListType.X,
                                         negate=True)
                    ef = spool.tile([P, e], f32, tag="ef", name="ef")
                    nc.scalar.activation(ef, lg, AF.Exp, bias=nm)
                    # top-8 straight on the unnormalized exps: selection order
                    # is scale-invariant and tv/sum(tv) cancels the softmax
                    # normalization
                    tv = spool.tile([P, 8], f32, tag="tv", name="tv")
                    nc.vector.max(out=tv, in_=ef)
                    ti = spool.tile([P, 8], u32, tag="ti", name="ti")
                    nc.vector.max_index(out=ti, in_max=tv, in_values=ef)

                    s8 = spool.tile([P, 1], f32, tag="s8", name="s8")
                    nc.vector.reduce_sum(out=s8, in_=tv, axis=mybir.AxisListType.X)
                    r8 = spool.tile([P, 1], f32, tag="r8", name="r8")
                    nc.vector.reciprocal(r8, s8)
                    wgt = spool.tile([P, 8], f16, tag="wgt", name="wgt")
                    nc.scalar.activation(wgt, tv, AF.Copy, scale=r8)

                    # subtile-local row index = p*64 + expert (int16-safe)
                    idx16 = spool.tile([P, 8], i16, tag="idx16", name="idx16")
                    nc.vector.tensor_tensor(
                        out=idx16, in0=ti,
                        in1=cst["rb64"].to_broadcast([P, 8]), op=OP.add)
                    # bounce leg A: contiguous p-major dump to DRAM (16-byte
                    # runs per partition -- NO tiny DRAM writes: 2-byte
                    # scattered writes RMW at the HBM controller and wedge
                    # the shared SDMA lanes for tens of us)
                    pD = dpool.tile([TOPK * P], i16, tag="pD", name="pD")
                    nc.sync.dma_start(
                        out=pD.rearrange("(p j) -> p j", p=P), in_=idx16)
                    state.append({"bg": bg, "wgt": wgt, "pD": pD})
                return state

            def routing_p2(state):
                """bounce legs B: four scattered-READ DMAs (reads don't RMW)
                fill the wrapped idx block [pi, j*8+po] = pD[(po*16+pi)*8+j]
                into the four 16-partition stripes the queue-0/1 Q7 cpu
                pairs read (partitions 0-63). All four depend only on A."""
                for st in state:
                    idxs_sb = cst[f"idxs_{st['bg'] % 3}"]
                    src = st["pD"].rearrange("(po pi j) -> pi j po",
                                             po=8, pi=16)
                    for m in range(4):
                        nc.sync.dma_start(
                            out=idxs_sb[16 * m:16 * (m + 1), :], in_=src)
                    st["idxs"] = idxs_sb

            def routing_p3(state):
                """half-gathers on rotating SWDGE queues + weighted sum."""
                for st in state:
                    bg, wgt, idxs_sb = st["bg"], st["wgt"], st["idxs"]
                    sel = selpool.tile([P, TOPK, c], f16, tag="sel", name="sel")
                    mt = selpool.tile([P, TOPK * c], f16, tag="mt", name="mt")
                    mt3 = mt.rearrange("p (k c) -> p k c", c=c)
                    wb = wgt.to_broadcast([P, TOPK, c])
                    pNsub = pN[bg * P * e:(bg + 1) * P * e, :]
                    for h in range(2):
                        # queue h: its Q7 pair reads idx partitions 32h..32h+31
                        nc.gpsimd.dma_gather(
                            sel[:, 4 * h:4 * h + 4, :], pNsub,
                            idxs_sb[:, 32 * h:32 * (h + 1)],
                            TOPK * P // 2, TOPK * P // 2, c,
                            queue_num=h)
                    for h in range(2):
                        nc.vector.tensor_tensor(
                            out=mt3[:, 4 * h:4 * h + 4, :],
                            in0=sel[:, 4 * h:4 * h + 4, :],
                            in1=wb[:, 4 * h:4 * h + 4, :], op=OP.mult)
                    for q in range(4):
                        nc.vector.tensor_add(
                            mt[:, 2 * q * c:(2 * q + 1) * c],
                            mt[:, 2 * q * c:(2 * q + 1) * c],
                            mt[:, (2 * q + 1) * c:(2 * q + 2) * c])
                    nc.vector.tensor_add(mt[:, :c], mt[:, :c], mt[:, 2 * c:3 * c])
                    nc.vector.tensor_add(mt[:, 4 * c:5 * c], mt[:, 4 * c:5 * c],
                                         mt[:, 6 * c:7 * c])
                    acc = opool.tile([P, c], f16, tag=f"acc_{bg}",
                                     name=f"acc_{bg}")
                    nc.vector.tensor_add(acc, mt[:, :c], mt[:, 4 * c:5 * c])
                    acc_tiles.append((bg, acc))

            nc.__enter_lp = nc.allow_low_precision(
                reason="fp16 weighted-sum tree of fp16 gathers; output "
                       "stores are fp16 regardless")
            nc.__enter_lp.__enter__()

            col0 = 0
            pending = None
            p1state = None
            for n, nt in enumerate(tiles):
                toff = KC * col0  # column offset of this tile's block in pTr
                ps_h1 = [psh1.tile([P, nt], f32, tag=f"h1_{m}", name=f"ps_h1_{m}")
                         for m in range(MC)]
                for g in range(KG):
                    xt = xpool.tile([P, kper, nt], f16, tag="xt", name="xt")
                    if n == 0 and g == 0:
                        # per-chunk DMAs so the first matmul only waits for
                        # chunk 0 of xt and W1, not the whole group
                        wt = w1pool.tile([P, kper, h1], f16, tag="w1_0",
                                         name="w1_0")
                        w1_tiles[0] = wt
                        for cc in range(kper):
                            nc.sync.dma_start(
                                out=xt[:, cc, :],
                                in_=pTr[:, toff + cc * nt:toff + (cc + 1) * nt])
                            nc.sync.dma_start(
                                out=wt[:, cc, :],
                                in_=w1r[:, cc * h1:(cc + 1) * h1])
                    else:
                        nc.sync.dma_start(
                            out=xt,
                            in_=pTr[:, toff + g * kper * nt:
                                    toff + (g + 1) * kper * nt]
                            .rearrange("p (c b) -> p c b", c=kper))
                        if n == 0:
                            wt = w1pool.tile([P, kper, h1], f16, tag=f"w1_{g}",
                                             name=f"w1_{g}")
                            nc.sync.dma_start(
                                out=wt,
                                in_=w1r[:, g * kper * h1:(g + 1) * kper * h1]
                                .rearrange("p (c h) -> p c h", c=kper))
                            w1_tiles[g] = wt
                    wt = w1_tiles[g]
                    for cc in range(kper):
                        for m in range(MC):
                            nc.tensor.matmul(
                                out=ps_h1[m], lhsT=wt[:, cc, m * P:(m + 1) * P],
                                rhs=xt[:, cc, :],
                                start=(g == 0 and cc == 0),
                                stop=(g == KG - 1 and cc == kper - 1))
                    if n == 0 and g == 2:
                        emit_consts()
                    if pending is not None:
                        # previous tile's routing, spread across this tile's
                        # k-group stream (phases separated by x-group DMAs on
                        # the sync queue so no queue waits at its head)
                        if g == 1:
                            p1state = routing_p1(*pending)
                        elif g == 3:
                            routing_p2(p1state)
                        elif g == 5:
                            routing_p3(p1state)
                            pending = None
                            p1state = None

                pending = (nt, col0, ps_h1)
                col0 += nt

            p1state = routing_p1(*pending)
            routing_p2(p1state)
            routing_p3(p1state)

            nc.__enter_lp.__exit__(None, None, None)

            # output stores, all at the tail of the in-order sync queue: by
            # the time the queue drains the xt stream, the early acc tiles
            # are long done
            for bg, acc in acc_tiles:
                nc.sync.dma_start(out=out[bg * P:(bg + 1) * P, :], in_=acc)

    nc.compile()
    return nc


def _prep_core_inputs(flat, W1b, w1r_, w2r_, w3_, b1, b2, b3, core,
                      tiles=TILES, kper=KPER):
    KC = D // P
    shard = flat[core * BS:(core + 1) * BS]                    # (BS, D)
    hf = shard.astype(np.float16)                              # (BS, D)
    blocks = []
    col0 = 0
    for nt in tiles:
        blk = hf[col0:col0 + nt, :].T                          # (D, nt)
        blocks.append(np.ascontiguousarray(
            blk.reshape(KC, P, nt).transpose(1, 0, 2).reshape(P, KC * nt)))
        col0 += nt
    pTr = np.concatenate(blocks, axis=1)                       # (P, KC*BS)
    pN = np.ascontiguousarray(hf).reshape(BS * E, C)
    b1r = np.ascontiguousarray(b1.reshape(H1 // P, P).T)
    b2r = np.ascontiguousarray(b2.reshape(H2, 1))
    b3r = np.ascontiguousarray(np.broadcast_to(b3, (P, E)))
    return {"pTr": pTr, "pN": pN, "w1r": w1r_, "w2r": w2r_, "w3": w3_,
            "b1r": b1r, "b2r": b2r, "b3r": b3r}


def _prep_weights(W1, W2, W3):
    KC = D // P
    w1r_ = np.ascontiguousarray(
        W1.astype(np.float16).reshape(KC, P, H1)
        .transpose(1, 0, 2).reshape(P, KC * H1))
    w2r_ = np.ascontiguousarray(
        W2.reshape(H1 // P, P, H2).transpose(1, 0, 2).reshape(P, -1))
    w3_ = np.ascontiguousarray(W3)
    return w1r_, w2r_, w3_


def kernel(expert_probs, W1, b1, W2, b2, W3, b3):
    from concourse.bass_utils import run_bass_kernel_spmd

    expert_probs = np.asarray(expert_probs, dtype=np.float32)
    W1 = np.asarray(W1, dtype=np.float32)
    W2 = np.asarray(W2, dtype=np.float32)
    W3 = np.asarray(W3, dtype=np.float32)
    b1 = np.asarray(b1, dtype=np.float32)
    b2 = np.asarray(b2, dtype=np.float32)
    b3 = np.asarray(b3, dtype=np.float32)

    if "nc" not in _BUILD_CACHE:
        _BUILD_CACHE["nc"] = build_moe_nc()
    nc = _BUILD_CACHE["nc"]

    flat = expert_probs.reshape(B, D)
    w1r_, w2r_, w3_ = _prep_weights(W1, W2, W3)
    in_maps = [_prep_core_inputs(flat, None, w1r_, w2r_, w3_, b1, b2, b3, cid)
               for cid in range(NCORES)]
    res = run_bass_kernel_spmd(nc, in_maps, core_ids=list(range(NCORES)))
    out = np.concatenate([res.results[cid]["out"] for cid in range(NCORES)], axis=0)
    return out.astype(np.float32)  # device stores fp16; ~1e-4 rel quantization
